# revision 1
# baseline (speedup 1.0000x reference)
"""Trainium2 Bass kernel for nn_EnhancedDDGAttention.

Sharding: data-parallel over the batch axis N=8 -> one batch element per
NeuronCore (8 cores). Each core runs an identical program on its slice;
weights are replicated. Host-side prep only reshapes/transposes inputs into
PE-friendly layouts (no FLOPs moved off-device).

Per-core pipeline (L=512, D=256, H=16, QK=V=32, OUT=256):
  1. projections:  qT/kT = (Wq|Wk)^T x^T  (+ spatial encoding on q),
     Vaug = [v | pos_CB | 1] row-masked by the key mask  (kills the -INF
     logit bias: masked keys contribute exactly 0 to numerator and denom)
  2. per head: S^T = k_h q_h^T in [key, query] layout -> E = exp(S^T) (no
     max-subtraction; logits are O(3)) -> alpha@Vaug via col-tiled matmuls
     giving feat_node^T rows, unnormalized apb rows and the softmax denom
  3. normalize, spatial features (dist / frame-rotated pts / dir / atan2)
     in [query, head] layout, transpose back to feature-major
  4. output MLP + residual + layernorm
"""

import os
from contextlib import ExitStack

import numpy as np

import concourse.bass as bass
import concourse.tile as tile
from concourse import bacc
from concourse import mybir
from concourse.bass_utils import run_bass_kernel_spmd
from concourse.masks import make_identity

N, L, D = 8, 512, 256
H, QK, V = 16, 32, 32
OUT = 256
PI = 3.14159265358979323846

f32 = mybir.dt.float32
f32r = mybir.dt.float32r

AF = mybir.ActivationFunctionType
ALU = mybir.AluOpType
AX = mybir.AxisListType

# fp32r runs 4x faster on the PE (1 cycle/row at N>=256), but walrus requires
# operands to be produced with fp32r dtype (rounded); enable via KERNEL_F32R=1.
USE_F32R = os.environ.get("KERNEL_F32R", "1") == "1"
MMDT = f32r if USE_F32R else f32


def _mm(ap):
    return ap


def build_program():
    KGROUPS = int(os.environ.get("KGROUPS", "4"))
    KVMM = os.environ.get("KVMM", "1") == "1"
    KPOST = os.environ.get("KPOST", "1") == "1"
    KSPATIAL = os.environ.get("KSPATIAL", "1") == "1"
    KMLP = os.environ.get("KMLP", "1") == "1"
    nc = bacc.Bacc()

    def inp(name, shape, dt=f32):
        return nc.declare_dram_parameter(name, list(shape), dt, isOutput=False)

    xT_d = inp("xT", (D, L), MMDT)
    x_d = inp("x", (L, D))
    posCAT_d = inp("posCAT", (3, L), MMDT)
    posCA_d = inp("posCA", (L, 3))
    posCB_d = inp("posCB", (L, 3))
    frame9_d = inp("frame9", (L, 9))
    maskpm_d = inp("maskpm", (128, 4))
    Wq_d = inp("Wq", (D, H * QK), MMDT)
    Wk_d = inp("Wk", (D, H * QK), MMDT)
    Wv_d = inp("Wv", (D, H * V), MMDT)
    seW1_d = inp("seW1", (3, QK), MMDT)
    seb1_d = inp("seb1", (QK, 1))
    seW2_d = inp("seW2", (QK, H * QK), MMDT)
    seb2pm_d = inp("seb2pm", (128, 4))
    otW1_d = inp("otW1", (H * V + 128, OUT * 2), MMDT)
    otb1pm_d = inp("otb1pm", (128, 4))
    otW2_d = inp("otW2", (OUT * 2, OUT), MMDT)
    otb2B_d = inp("otb2B", (128, OUT))
    lngB_d = inp("lngB", (128, OUT))
    lnbB_d = inp("lnbB", (128, OUT))
    selg_d = inp("selg", (4, 128), MMDT)
    out_d = nc.declare_dram_parameter("out", [L, OUT], f32, isOutput=True)

    with tile.TileContext(nc) as tc, ExitStack() as ctx:
        consts = ctx.enter_context(tc.tile_pool(name="consts", bufs=1))
        wpool = ctx.enter_context(tc.tile_pool(name="weights", bufs=1))
        work = ctx.enter_context(tc.tile_pool(name="work", bufs=1))

        # ---- constants -------------------------------------------------
        ident = consts.tile([128, 128], f32)
        make_identity(nc, ident)
        zeros512 = consts.tile([128, 512], f32)
        nc.gpsimd.memset(zeros512, 0.0)
        # column-group selector: selg[q, p] = 1 iff q == p // 32 (host const)
        selg = consts.tile([4, 128], MMDT)
        epsln = consts.tile([128, 1], f32)
        nc.vector.memset(epsln, 1e-5)

        # ---- input DMAs ------------------------------------------------
        def dma(t, src):
            if not isinstance(src, bass.AP):
                src = src[:, :]
            nc.sync.dma_start(out=t, in_=src)

        xTs = []
        for i in range(2):
            t = wpool.tile([128, 512], MMDT, name=f"xT{i}")
            dma(t, xT_d[128 * i : 128 * (i + 1), :])
            xTs.append(t)
        Wqs, Wks, Wvs = [], [], []
        for i in range(2):
            t = wpool.tile([128, 512], MMDT, name=f"Wq{i}")
            dma(t, Wq_d[128 * i : 128 * (i + 1), :])
            Wqs.append(t)
            t = wpool.tile([128, 512], MMDT, name=f"Wk{i}")
            dma(t, Wk_d[128 * i : 128 * (i + 1), :])
            Wks.append(t)
            t = wpool.tile([128, 512], MMDT, name=f"Wv{i}")
            dma(t, Wv_d[128 * i : 128 * (i + 1), :])
            Wvs.append(t)
        posCATs = wpool.tile([3, 512], MMDT)
        dma(posCATs, posCAT_d[:, :])
        dma(selg, selg_d[:, :])
        seW1s = wpool.tile([3, 32], MMDT)
        dma(seW1s, seW1_d[:, :])
        seb1s = wpool.tile([32, 1], f32)
        dma(seb1s, seb1_d[:, :])
        seW2s = wpool.tile([32, 512], MMDT)
        dma(seW2s, seW2_d[:, :])
        seb2s = wpool.tile([128, 4], f32)
        dma(seb2s, seb2pm_d)
        maskpm = wpool.tile([128, 4], f32)
        dma(maskpm, maskpm_d)
        maskr = wpool.tile([128, 4], MMDT)
        nc.vector.tensor_copy(out=maskr, in_=maskpm)
        posCBn, posCAn, frame9, xn = [], [], [], []
        for c in range(4):
            t = wpool.tile([128, 3], f32, name=f"posCB{c}")
            dma(t, posCB_d[128 * c : 128 * (c + 1), :])
            posCBn.append(t)
            t = wpool.tile([128, 3], f32, name=f"posCA{c}")
            dma(t, posCA_d[128 * c : 128 * (c + 1), :])
            posCAn.append(t)
            t = wpool.tile([128, 9], f32, name=f"frame9{c}")
            dma(t, frame9_d[128 * c : 128 * (c + 1), :])
            frame9.append(t)
            t = wpool.tile([128, 256], f32, name=f"xn{c}")
            dma(t, x_d[128 * c : 128 * (c + 1), :])
            xn.append(t)
        otW1s = []
        for i in range(5):
            t = wpool.tile([128, 512], MMDT, name=f"otW1_{i}")
            dma(t, otW1_d[128 * i : 128 * (i + 1), :])
            otW1s.append(t)
        otb1s = wpool.tile([128, 4], f32)
        dma(otb1s, otb1pm_d)
        otW2s = []
        for i in range(4):
            t = wpool.tile([128, 256], MMDT, name=f"otW2_{i}")
            dma(t, otW2_d[128 * i : 128 * (i + 1), :])
            otW2s.append(t)
        otb2B = wpool.tile([128, 256], f32)
        dma(otb2B, otb2B_d)
        lngB = wpool.tile([128, 256], f32)
        dma(lngB, lngB_d)
        lnbB = wpool.tile([128, 256], f32)
        dma(lnbB, lnbB_d)

        # invrow = 1 - mask  (per-partition, one col per l-chunk)
        invrow = consts.tile([128, 4], f32)
        nc.vector.tensor_scalar(
            out=invrow, in0=maskpm, scalar1=-1.0, scalar2=1.0, op0=ALU.mult, op1=ALU.add
        )
        # pos_CA * rowmask, natural layout
        posCAm = []
        for c in range(4):
            t = work.tile([128, 3], f32, name=f"posCAm{c}")
            nc.vector.tensor_scalar(
                out=t, in0=posCAn[c], scalar1=maskpm[:, c : c + 1], scalar2=None,
                op0=ALU.mult,
            )
            posCAm.append(t)
        # xb = x + ot_b2 * rowmask (pre-add for the final residual)
        xb = []
        for c in range(4):
            t = work.tile([128, 256], f32, name=f"xb{c}")
            nc.vector.scalar_tensor_tensor(
                out=t, in0=otb2B, scalar=maskpm[:, c : c + 1], in1=xn[c],
                op0=ALU.mult, op1=ALU.add,
            )
            xb.append(t)

        # ---- projections ----------------------------------------------
        qT, kT, Vaug = [], [], []
        with tc.tile_pool(name="psproj", bufs=2, space="PSUM") as psproj:
            # spatial encoding: r1 = relu(se_W1^T @ posCA^T + b1)
            ps_r1 = psproj.tile([32, 512], f32)
            nc.tensor.matmul(
                out=ps_r1, lhsT=_mm(seW1s), rhs=_mm(posCATs), start=True, stop=True
            )
            r1 = work.tile([32, 512], MMDT)
            nc.scalar.activation(
                out=r1, in_=ps_r1, func=AF.Relu, bias=seb1s, scale=1.0
            )

            for mc in range(4):
                ps_q = psproj.tile([128, 512], f32, tag="ps_q")
                for kc in range(2):
                    nc.tensor.matmul(
                        out=ps_q,
                        lhsT=_mm(Wqs[kc][:, 128 * mc : 128 * (mc + 1)]),
                        rhs=_mm(xTs[kc]),
                        start=(kc == 0),
                        stop=False,
                    )
                nc.tensor.matmul(
                    out=ps_q,
                    lhsT=_mm(seW2s[:, 128 * mc : 128 * (mc + 1)]),
                    rhs=_mm(r1),
                    start=False,
                    stop=True,
                )
                t = work.tile([128, 512], MMDT, name=f"qT{mc}")
                nc.vector.tensor_scalar(
                    out=t, in0=ps_q, scalar1=seb2s[:, mc : mc + 1], scalar2=None,
                    op0=ALU.add,
                )
                qT.append(t)

                ps_k = psproj.tile([128, 512], f32, tag="ps_k")
                for kc in range(2):
                    nc.tensor.matmul(
                        out=ps_k,
                        lhsT=_mm(Wks[kc][:, 128 * mc : 128 * (mc + 1)]),
                        rhs=_mm(xTs[kc]),
                        start=(kc == 0),
                        stop=(kc == 1),
                    )
                t = work.tile([128, 512], MMDT, name=f"kT{mc}")
                nc.vector.tensor_copy(out=t, in_=ps_k)
                kT.append(t)

            for lc in range(4):
                ps_v = psproj.tile([128, 512], f32, tag="ps_v")
                for kc in range(2):
                    nc.tensor.matmul(
                        out=ps_v,
                        lhsT=_mm(xTs[kc][:, 128 * lc : 128 * (lc + 1)]),
                        rhs=_mm(Wvs[kc]),
                        start=(kc == 0),
                        stop=(kc == 1),
                    )
                va = work.tile([128, H * 36], MMDT, name=f"Vaug{lc}")
                vav = va.rearrange("p (h j) -> p h j", j=36)
                mcol = maskpm[:, lc : lc + 1]
                # value columns, zeroed on masked keys
                nc.vector.tensor_scalar(
                    out=vav[:, :, 0:32],
                    in0=ps_v.rearrange("p (h j) -> p h j", j=32),
                    scalar1=mcol, scalar2=None, op0=ALU.mult,
                )
                # pos_CB columns (masked), replicated across heads
                pcbm = work.tile([128, 3], MMDT, tag="pcbm")
                nc.vector.tensor_scalar(
                    out=pcbm, in0=posCBn[lc], scalar1=mcol, scalar2=None, op0=ALU.mult
                )
                nc.gpsimd.tensor_copy(
                    out=vav[:, :, 32:35],
                    in_=pcbm.unsqueeze(1).broadcast_to([128, H, 3]),
                )
                # ones column -> softmax denominator (masked)
                nc.gpsimd.tensor_copy(
                    out=vav[:, :, 35:36],
                    in_=maskr[:, lc : lc + 1].unsqueeze(1).broadcast_to([128, H, 1]),
                )
                Vaug.append(va)

        # ---- attention -------------------------------------------------
        featT = [work.tile([128, 512], MMDT, name=f"featT{g}") for g in range(5)]
        apb = [work.tile([128, 64], f32, name=f"apb{c}") for c in range(4)]

        with (
            tc.tile_pool(name="psS", bufs=1, space="PSUM") as psS_pool,
            tc.tile_pool(name="psV", bufs=4, space="PSUM") as psV_pool,
            tc.tile_pool(name="psT", bufs=1, space="PSUM") as psT_pool,
            tc.tile_pool(name="Epool", bufs=6) as E_pool,
            tc.tile_pool(name="gwork", bufs=2) as gwork,
        ):
            psG_pool = psT_pool
            stageUs = {}

            def emit_heads(g):
                psVs = []
                for s in range(4):
                    h = 4 * g + s
                    ch, r = h // 4, (h % 4) * 32
                    Es = []
                    for half in range(2):
                        ps_S = psS_pool.tile([128, 1024], f32, tag="ps_S")
                        for kcl in range(2):
                            kc = 2 * half + kcl
                            nc.tensor.matmul(
                                out=ps_S[:, 512 * kcl : 512 * (kcl + 1)],
                                lhsT=_mm(kT[ch][r : r + 32, 128 * kc : 128 * (kc + 1)]),
                                rhs=_mm(qT[ch][r : r + 32, :]),
                                start=True,
                                stop=True,
                                tile_position=(r, 0),
                            )
                        E = E_pool.tile([128, 1024], MMDT, tag="E")
                        nc.scalar.activation(out=E, in_=ps_S, func=AF.Exp)
                        Es.append(E)
                    ps_v = psV_pool.tile([36, 512], f32, tag="ps_v")
                    psVs.append(ps_v)
                    for kc in range(4):
                        if not KVMM:
                            break
                        Eh = Es[kc // 2][:, 512 * (kc % 2) : 512 * (kc % 2 + 1)]
                        vav = Vaug[kc].rearrange("p (h j) -> p h j", j=36)
                        nc.tensor.matmul(
                            out=ps_v,
                            lhsT=_mm(vav[:, h, 0:36]),
                            rhs=_mm(Eh),
                            start=(kc == 0),
                            stop=(kc == 3),
                        )
                    if not KVMM and g == 0 and s == 3:
                        dbg = gwork.tile([128, 256], f32, tag="dbg")
                        nc.vector.tensor_copy(out=dbg, in_=Es[0][:, 0:256])
                        nc.sync.dma_start(out=out_d[0:128, :], in_=dbg)
                    if not KPOST and KVMM and g == 0 and s == 3:
                        dbg = gwork.tile([128, 256], f32, tag="dbg")
                        nc.vector.tensor_copy(out=dbg, in_=ps_v[0:32, 0:256])
                        nc.sync.dma_start(out=out_d[0:128, :], in_=dbg)
                    if KPOST:
                        if s == 0:
                            stageUs[g] = gwork.tile(
                                [128, 512], f32, tag="stageU", name=f"stageU{g}"
                            )
                        if s % 2 == 0:
                            nc.scalar.activation(
                                out=stageUs[g][32 * s : 32 * s + 4, :],
                                in_=ps_v[32:36, :], func=AF.Copy,
                            )
                            nc.scalar.activation(
                                out=featT[g][32 * s : 32 * s + 32, :],
                                in_=ps_v[0:32, :], func=AF.Copy,
                            )
                        else:
                            nc.vector.tensor_copy(
                                out=stageUs[g][32 * s : 32 * s + 4, :],
                                in_=ps_v[32:36, :],
                            )
                            nc.vector.tensor_copy(
                                out=featT[g][32 * s : 32 * s + 32, :],
                                in_=ps_v[0:32, :],
                            )

            def emit_post(g):
                if not KPOST:
                    return
                KPLVL = int(os.environ.get("KPLVL", "7"))
                stageU = stageUs[g]
                if KPLVL < 2:
                    return
                # full-K [128,128] transposes: row-group spans all rows, so no
                # later LDWEIGHTS can be reorder-pulled ahead of them (HW
                # pulls LDWEIGHTS past in-flight MMs when row groups don't
                # conflict -- fatal across a transpose-mode boundary)
                ps_t = psT_pool.tile([128, 512], f32, tag="ps_t")
                for c in range(4):
                    nc.tensor.transpose(
                        out=ps_t[:, 128 * c : 128 * (c + 1)],
                        in_=stageU[:, 128 * c : 128 * (c + 1)],
                        identity=ident,
                    )
                if KPLVL < 3:
                    return
                for c in range(4):
                    nc.vector.tensor_copy(
                        out=apb[c][:, 16 * g : 16 * (g + 1)].rearrange(
                            "p (s j) -> p s j", j=4
                        ),
                        in_=ps_t[:, 128 * c : 128 * (c + 1)].rearrange(
                            "p (s q) -> p s q", q=32
                        )[:, :, 0:4],
                    )
                if KPLVL < 4:
                    return

                # recip = rowmask / (s + (1 - rowmask)), in [l, (c, s)] layout
                rec = gwork.tile([128, 16], f32, tag="rec")
                recm = gwork.tile([128, 16], f32, tag="recm")
                for c in range(4):
                    av = apb[c].rearrange("p (h j) -> p h j", j=4)
                    nc.vector.tensor_scalar(
                        out=rec[:, 4 * c : 4 * c + 4],
                        in0=av[:, 4 * g : 4 * g + 4, 3],
                        scalar1=invrow[:, c : c + 1], scalar2=None, op0=ALU.add,
                    )
                nc.vector.reciprocal(out=recm, in_=rec)
                for c in range(4):
                    nc.vector.tensor_scalar(
                        out=recm[:, 4 * c : 4 * c + 4],
                        in0=recm[:, 4 * c : 4 * c + 4],
                        scalar1=maskpm[:, c : c + 1], scalar2=None, op0=ALU.mult,
                    )

                if KPLVL < 5:
                    return
                # transpose recip to [head-in-group, l] and broadcast to the
                # 32-row blocks of the feat_node chunk
                ps_rt = psG_pool.tile([4, 512], f32, tag="ps_g")
                for c in range(4):
                    nc.tensor.transpose(
                        out=ps_rt[:, 128 * c : 128 * (c + 1)],
                        in_=recm[:, 4 * c : 4 * c + 4],
                        identity=ident,
                    )
                recT = gwork.tile([4, 512], MMDT, tag="recT")
                nc.vector.tensor_copy(out=recT, in_=ps_rt)
                if KPLVL < 6:
                    return
                ps_b = psG_pool.tile([128, 512], f32, tag="ps_g")
                nc.tensor.matmul(
                    out=ps_b, lhsT=_mm(selg), rhs=_mm(recT), start=True, stop=True
                )
                recB = gwork.tile([128, 512], f32, tag="recB")
                nc.scalar.activation(out=recB, in_=ps_b, func=AF.Copy)
                nc.vector.tensor_tensor(
                    out=featT[g], in0=featT[g], in1=recB, op=ALU.mult
                )

                # normalize apb in place: apb*recip - posCA*rowmask
                for c in range(4 if KPLVL >= 7 else 0):
                    av = apb[c].rearrange("p (h j) -> p h j", j=4)
                    blk = av[:, 4 * g : 4 * g + 4, :]
                    nc.vector.tensor_tensor(
                        out=blk,
                        in0=blk,
                        in1=recm[:, 4 * c : 4 * c + 4]
                        .unsqueeze(2)
                        .broadcast_to([128, 4, 4]),
                        op=ALU.mult,
                    )
                    blk3 = av[:, 4 * g : 4 * g + 4, 0:3]
                    nc.vector.tensor_tensor(
                        out=blk3,
                        in0=blk3,
                        in1=posCAm[c].unsqueeze(1).broadcast_to([128, 4, 3]),
                        op=ALU.subtract,
                    )

            for g in range(KGROUPS):
                emit_heads(g)
                if g > 0:
                    emit_post(g - 1)
            if KGROUPS > 0:
                emit_post(KGROUPS - 1)

        if (KPOST and KGROUPS > 0 and not KMLP
                and int(os.environ.get("KPLVL", "7")) >= 6):
            nc.sync.dma_start(out=out_d[128:256, :],
                              in_=featT[KGROUPS - 1][:, 0:256])
        if KGROUPS == 0:
            nc.sync.dma_start(out=out_d[0:128, :], in_=qT[0][:, 0:256])
        # ---- spatial features ------------------------------------------
        spat = [work.tile([128, 128], f32, name=f"spat{c}") for c in range(4)]
        qs = work.tile([128, 64], f32)
        axs = work.tile([128, 64], f32)
        ays = work.tile([128, 64], f32)
        with tc.tile_pool(name="spwork", bufs=1) as spw:
            if not KSPATIAL:
                spatial_off = True
            else:
                spatial_off = False
            t16a = [spw.tile([128, 16], f32, name=f"t16a{c}") for c in range(4)]
            t16b = [spw.tile([128, 16], f32, name=f"t16b{c}") for c in range(4)]
            sq3 = [spw.tile([128, 48], f32, name=f"sq3{c}") for c in range(4)]
            for c in range(4):
                if spatial_off:
                    break
                av = apb[c].rearrange("p (h j) -> p h j", j=4)
                A = [av[:, :, j] for j in range(3)]
                pts = spat[c][:, 0:48].rearrange("p (h i) -> p h i", i=3)
                fr = frame9[c]
                for i in range(3):
                    f = lambda j: fr[:, 3 * i + j : 3 * i + j + 1]
                    nc.gpsimd.tensor_scalar(
                        out=t16a[c], in0=A[0], scalar1=f(0), scalar2=None, op0=ALU.mult
                    )
                    nc.gpsimd.tensor_scalar(
                        out=t16b[c], in0=A[1], scalar1=f(1), scalar2=None, op0=ALU.mult
                    )
                    nc.gpsimd.tensor_tensor(
                        out=t16a[c], in0=t16a[c], in1=t16b[c], op=ALU.add
                    )
                    nc.gpsimd.tensor_scalar(
                        out=t16b[c], in0=A[2], scalar1=f(2), scalar2=None, op0=ALU.mult
                    )
                    nc.gpsimd.tensor_tensor(
                        out=pts[:, :, i], in0=t16a[c], in1=t16b[c], op=ALU.add
                    )
                # dist = |apb|
                av3 = av[:, :, 0:3]
                nc.gpsimd.tensor_tensor(out=sq3[c], in0=av3, in1=av3, op=ALU.mult)
                nc.vector.tensor_reduce(
                    out=t16a[c],
                    in_=sq3[c].rearrange("p (h i) -> p h i", i=3),
                    axis=AX.X, op=ALU.add,
                )
                nc.scalar.activation(
                    out=spat[c][:, 48:64], in_=t16a[c], func=AF.Sqrt
                )
                # dir = pts / (|pts| + 1e-10)
                nc.gpsimd.tensor_tensor(out=sq3[c], in0=pts, in1=pts, op=ALU.mult)
                nc.vector.tensor_reduce(
                    out=t16a[c],
                    in_=sq3[c].rearrange("p (h i) -> p h i", i=3),
                    axis=AX.X, op=ALU.add,
                )
                nc.scalar.activation(out=t16b[c], in_=t16a[c], func=AF.Sqrt)
                nc.vector.tensor_scalar(
                    out=t16b[c], in0=t16b[c], scalar1=1e-10, scalar2=None, op0=ALU.add
                )
                nc.vector.reciprocal(out=t16b[c], in_=t16b[c])
                nc.gpsimd.tensor_tensor(
                    out=spat[c][:, 64:112].rearrange("p (h i) -> p h i", i=3),
                    in0=pts,
                    in1=t16b[c].unsqueeze(2).broadcast_to([128, 16, 3]),
                    op=ALU.mult,
                )
                # atan2 range reduction: a = min(|x|,|y|) / max(|x|,|y|)
                # (the ACT Arctan table only accepts [-pi/2, pi/2])
                px, py = pts[:, :, 0], pts[:, :, 1]
                axv = axs[:, 16 * c : 16 * (c + 1)]
                ayv = ays[:, 16 * c : 16 * (c + 1)]
                nc.gpsimd.tensor_scalar(
                    out=t16a[c], in0=px, scalar1=-1.0, scalar2=None, op0=ALU.mult
                )
                nc.vector.tensor_tensor(out=axv, in0=px, in1=t16a[c], op=ALU.max)
                nc.gpsimd.tensor_scalar(
                    out=t16a[c], in0=py, scalar1=-1.0, scalar2=None, op0=ALU.mult
                )
                nc.vector.tensor_tensor(out=ayv, in0=py, in1=t16a[c], op=ALU.max)
                nc.vector.tensor_tensor(out=t16a[c], in0=axv, in1=ayv, op=ALU.min)
                nc.gpsimd.tensor_scalar(
                    out=t16b[c], in0=axv, scalar1=1e-38, scalar2=None, op0=ALU.add
                )
                nc.vector.tensor_tensor(out=t16b[c], in0=t16b[c], in1=ayv, op=ALU.max)
                nc.vector.reciprocal(out=t16b[c], in_=t16b[c])
                nc.gpsimd.tensor_tensor(
                    out=qs[:, 16 * c : 16 * (c + 1)], in0=t16a[c], in1=t16b[c],
                    op=ALU.mult,
                )
            for c in range(4):
                if spatial_off:
                    break
                pts = spat[c][:, 0:48].rearrange("p (h i) -> p h i", i=3)
                px, py = pts[:, :, 0], pts[:, :, 1]
                axv = axs[:, 16 * c : 16 * (c + 1)]
                ayv = ays[:, 16 * c : 16 * (c + 1)]
                ang = spat[c][:, 112:128]
                nc.scalar.activation(
                    out=ang, in_=qs[:, 16 * c : 16 * (c + 1)], func=AF.Arctan
                )
                # t += (|y|>|x|) * (pi/2 - 2t); t += (x<0) * (pi - 2t)
                nc.vector.tensor_tensor(out=t16a[c], in0=ayv, in1=axv, op=ALU.is_gt)
                nc.gpsimd.tensor_scalar(
                    out=t16b[c], in0=ang, scalar1=-2.0, scalar2=PI / 2,
                    op0=ALU.mult, op1=ALU.add,
                )
                nc.gpsimd.tensor_tensor(
                    out=t16a[c], in0=t16a[c], in1=t16b[c], op=ALU.mult
                )
                nc.gpsimd.tensor_tensor(out=ang, in0=ang, in1=t16a[c], op=ALU.add)
                nc.gpsimd.tensor_scalar(
                    out=t16a[c], in0=px, scalar1=0.0, scalar2=None, op0=ALU.is_lt
                )
                nc.gpsimd.tensor_scalar(
                    out=t16b[c], in0=ang, scalar1=-2.0, scalar2=PI,
                    op0=ALU.mult, op1=ALU.add,
                )
                nc.gpsimd.tensor_tensor(
                    out=t16a[c], in0=t16a[c], in1=t16b[c], op=ALU.mult
                )
                nc.gpsimd.tensor_tensor(out=ang, in0=ang, in1=t16a[c], op=ALU.add)
                # t *= sign(y):  2*t*(y>=0) - t
                nc.gpsimd.tensor_scalar(
                    out=t16a[c], in0=py, scalar1=0.0, scalar2=None, op0=ALU.is_ge
                )
                nc.gpsimd.tensor_tensor(
                    out=t16a[c], in0=t16a[c], in1=ang, op=ALU.mult
                )
                nc.gpsimd.tensor_scalar(
                    out=t16a[c], in0=t16a[c], scalar1=2.0, scalar2=None, op0=ALU.mult
                )
                nc.gpsimd.tensor_tensor(out=ang, in0=t16a[c], in1=ang, op=ALU.subtract)

        # transpose spatial features into featT[4]
        if KSPATIAL:
          with tc.tile_pool(name="psSp", bufs=1, space="PSUM") as psSp_pool:
            ps_sp = psSp_pool.tile([128, 512], f32)
            for c in range(4):
                nc.tensor.transpose(
                    out=ps_sp[:, 128 * c : 128 * (c + 1)], in_=spat[c], identity=ident
                )
            nc.vector.tensor_copy(out=featT[4], in_=ps_sp)
        # ---- output MLP + residual + layernorm -------------------------
        h1T = [work.tile([128, 512], MMDT, name=f"h1T{mc}") for mc in range(4)]
        with tc.tile_pool(name="psM", bufs=2, space="PSUM") as psM_pool:
            for mc in range(4 if KMLP else 0):
                ps_h = psM_pool.tile([128, 512], f32, tag="ps_h")
                for kc in range(5):
                    nc.tensor.matmul(
                        out=ps_h,
                        lhsT=_mm(otW1s[kc][:, 128 * mc : 128 * (mc + 1)]),
                        rhs=_mm(featT[kc]),
                        start=(kc == 0),
                        stop=(kc == 4),
                    )
                nc.vector.scalar_tensor_tensor(
                    out=h1T[mc], in0=ps_h, scalar=otb1s[:, mc : mc + 1], in1=zeros512,
                    op0=ALU.add, op1=ALU.max,
                )
            for lc in range(4 if KMLP else 0):
                ps_o = psM_pool.tile([128, 256], f32, tag="ps_o")
                for kc in range(4):
                    nc.tensor.matmul(
                        out=ps_o,
                        lhsT=_mm(h1T[kc][:, 128 * lc : 128 * (lc + 1)]),
                        rhs=_mm(otW2s[kc]),
                        start=(kc == 0),
                        stop=(kc == 3),
                    )
                y = work.tile([128, 256], f32, tag="y_ln")
                nc.vector.scalar_tensor_tensor(
                    out=y, in0=ps_o, scalar=maskpm[:, lc : lc + 1], in1=xb[lc],
                    op0=ALU.mult, op1=ALU.add,
                )
                stats = work.tile([128, 6], f32, tag="ln_stats")
                mv = work.tile([128, 2], f32, tag="ln_mv")
                nc.vector.bn_stats(out=stats, in_=y)
                nc.vector.bn_aggr(out=mv, in_=stats)
                sd = work.tile([128, 1], f32, tag="ln_sd")
                nc.scalar.activation(
                    out=sd, in_=mv[:, 1:2], func=AF.Sqrt, bias=epsln, scale=1.0
                )
                rstd = work.tile([128, 1], f32, tag="ln_rstd")
                nc.vector.reciprocal(out=rstd, in_=sd)
                z = work.tile([128, 256], f32, tag="z_out")
                nc.vector.scalar_tensor_tensor(
                    out=z, in0=y, scalar=mv[:, 0:1], in1=lngB,
                    op0=ALU.subtract, op1=ALU.mult,
                )
                nc.vector.scalar_tensor_tensor(
                    out=z, in0=z, scalar=rstd, in1=lnbB,
                    op0=ALU.mult, op1=ALU.add,
                )
                nc.sync.dma_start(out=out_d[128 * lc : 128 * (lc + 1), :], in_=z)

    nc.finalize()
    return nc


_cached = {}


def _get_program():
    if "nc" not in _cached:
        _cached["nc"] = build_program()
    return _cached["nc"]


def _prep_core_inputs(x, pos_CA, pos_CB, frame, mask, Wq, Wk, Wv, seW1, seb1,
                      seW2, seb2, otW1, otb1, otW2, otb2, lng, lnb):
    f = np.float32
    return {
        "xT": np.ascontiguousarray(x.T, dtype=f),
        "x": np.ascontiguousarray(x, dtype=f),
        "posCAT": np.ascontiguousarray(pos_CA.T, dtype=f),
        "posCA": np.ascontiguousarray(pos_CA, dtype=f),
        "posCB": np.ascontiguousarray(pos_CB, dtype=f),
        "frame9": np.ascontiguousarray(frame.reshape(L, 9), dtype=f),
        "maskpm": np.ascontiguousarray(
            mask.astype(f).reshape(4, 128).T
        ),
        "Wq": np.ascontiguousarray(Wq, dtype=f),
        "Wk": np.ascontiguousarray(Wk, dtype=f),
        "Wv": np.ascontiguousarray(Wv, dtype=f),
        "seW1": np.ascontiguousarray(seW1, dtype=f),
        "seb1": np.ascontiguousarray(seb1.reshape(QK, 1), dtype=f),
        "seW2": np.ascontiguousarray(seW2, dtype=f),
        "seb2pm": np.ascontiguousarray(seb2.astype(f).reshape(4, 128).T),
        "otW1": np.ascontiguousarray(otW1, dtype=f),
        "otb1pm": np.ascontiguousarray(otb1.astype(f).reshape(4, 128).T),
        "otW2": np.ascontiguousarray(otW2, dtype=f),
        "otb2B": np.ascontiguousarray(np.tile(otb2.astype(f), (128, 1))),
        "lngB": np.ascontiguousarray(np.tile(lng.astype(f), (128, 1))),
        "lnbB": np.ascontiguousarray(np.tile(lnb.astype(f), (128, 1))),
        "selg": np.ascontiguousarray(
            (np.arange(4)[:, None] == (np.arange(128)[None, :] // 32)).astype(f)
        ),
    }


def kernel(x, pos_CA, pos_CB, frame, mask,
           Wq, Wk, Wv, se_W1, se_b1, se_W2, se_b2,
           ot_W1, ot_b1, ot_W2, ot_b2, ln_g, ln_b):
    x = np.asarray(x, dtype=np.float32)
    pos_CA = np.asarray(pos_CA, dtype=np.float32)
    pos_CB = np.asarray(pos_CB, dtype=np.float32)
    frame = np.asarray(frame, dtype=np.float32)
    mask = np.asarray(mask)
    weights = [np.asarray(w, dtype=np.float32) for w in
               (Wq, Wk, Wv, se_W1, se_b1, se_W2, se_b2,
                ot_W1, ot_b1, ot_W2, ot_b2, ln_g, ln_b)]

    nc = _get_program()
    in_maps = [
        _prep_core_inputs(x[i], pos_CA[i], pos_CB[i], frame[i], mask[i], *weights)
        for i in range(N)
    ]
    res = run_bass_kernel_spmd(nc, in_maps, list(range(N)))
    _cached["last_results"] = res
    out = np.stack([res.results[i]["out"] for i in range(N)], axis=0)
    return out.astype(np.float32)

